# revision 58
# baseline (speedup 1.0000x reference)
"""DenseCRFLoss on 8 Trainium2 NeuronCores.

Math: loss = -W/N * sum_k s_k^T K s_k per image, K[p,q] = exp(-0.5*||f_p-f_q||^2),
f = (x/50, y/50, rgb/15) on the 64x64 downsampled image, P=4096 pixels.

Device strategy (per core, SPMD-uniform program; data assignment differs per core):
  - K is symmetric with unit diagonal: only the strict upper triangle is computed
    (2x saving); the diagonal term sum(s^2) is a separate cheap reduction.
  - The PxP exp argument is produced directly by one bf16 matmul: the feature
    vectors are hi/lo bf16-split (fp32-accurate dot products) and the -0.5*|f|^2
    row/column terms are folded in as extra contraction rows, so the PSUM tile
    holds -0.5*d^2 exactly and ScalarE applies a plain exp (no bias plumbing).
  - The quadratic form uses a second tiny matmul per block (s rows as stationary)
    accumulating u = sum_rows s*E in PSUM per quad of 4 row-tiles, then one DVE
    mult+reduce per quad dots u with s_cols into per-quad partial sums.
  - Work unit: "quad" = 4 [128x512] blocks of one (image, column-chunk). Each
    image yields 36 quads (triangle); each of the 8 cores gets 18 quads of one
    image (2 cores per image). Every core runs the identical instruction stream.
  - Diagonal-straddling blocks skip the below-diagonal columns entirely and
    compute the diagonal 128x128 subtile UNMASKED with half-weighted s rows:
    by subtile symmetry 2*(total partial sums) equals the full quadratic form
    including the diagonal, so no masking and no separate diag term is needed.
"""

import numpy as np
import ml_dtypes

WEIGHT = 2e-9
SIGMA_RGB = 15.0
SIGMA_XY = 100.0
SCALE = 0.5

NQ = 18          # quads per core
NB = NQ * 4      # blocks per core
STRADDLE_SLOTS = (14, 15, 16, 17)  # quad slots holding diagonal-straddling quads
NROWS = 24       # contraction rows of the feature stack

# Straddle-quad packed G/E layout: block j covers chunk cols [128j:512], width
# 512-128j; packed into 3 PSUM banks as j0:[0:512] j1:[512:896] j3:[896:1024]
# j2:[1024:1280] (no block crosses a bank boundary).
_S_OFF = (0, 512, 1024, 896)
_S_W = (512, 384, 256, 128)

_bf16 = ml_dtypes.bfloat16

_PROGRAM_CACHE = {}


def _build_program(reps=1, ablate=()):
    import concourse.bacc as bacc
    import concourse.tile as tile
    from concourse import mybir

    nc = bacc.Bacc("TRN2", target_bir_lowering=False)
    dt = mybir.dt

    feat = nc.dram_tensor("feat", [NROWS, 2, NQ, 512], dt.bfloat16, kind="ExternalInput")
    srows = nc.dram_tensor("srows", [128, NQ, 4, 4], dt.bfloat16, kind="ExternalInput")
    scols = nc.dram_tensor("scols", [2, NQ, 512], dt.bfloat16, kind="ExternalInput")
    out = nc.dram_tensor("out", [2, 19], dt.float32, kind="ExternalOutput")

    with tile.TileContext(nc) as tc:
        with (
            tc.tile_pool(name="consts", bufs=1) as consts,
            tc.tile_pool(name="gpsum", bufs=2, space="PSUM") as gpool,
            tc.tile_pool(name="upsum", bufs=2, space="PSUM") as upool,
            tc.tile_pool(name="epool", bufs=6) as epool,
            tc.tile_pool(name="scratch", bufs=4) as spool,
            tc.tile_pool(name="accp", bufs=4) as accp,
        ):
            # --- input DMAs (chunked so compute can start early) ---
            feat_sb = consts.tile([NROWS, 2, NQ, 512], dt.bfloat16)
            srows_sb = consts.tile([128, NQ, 4, 4], dt.bfloat16)
            scols_sb = consts.tile([2, NQ, 512], dt.bfloat16)
            stat_sb = feat_sb[:, 0]
            mov_sb = feat_sb[:, 1]

            # graded chunks, ordered by first use: tiny first chunk so
            # compute starts ASAP; straddle quads' chunks at their interleave
            chunks = [(0, 1), (1, 2), (2, 3), (3, 4), (14, 15), (4, 6), (6, 8),
                      (15, 16), (8, 10), (10, 12), (16, 17), (12, 14), (17, 18)]

            def feat_chunk(k):
                lo, hi = chunks[k]
                nc.sync.dma_start(out=feat_sb[:, :, lo:hi, :], in_=feat[:, :, lo:hi, :])

            feat_chunk(0)
            feat_chunk(1)
            nc.sync.dma_start(out=srows_sb, in_=srows[:, :, :, :])
            feat_chunk(2)
            feat_chunk(3)
            nc.sync.dma_start(out=scols_sb, in_=scols[:, :, :])
            for k in range(4, len(chunks)):
                feat_chunk(k)

            for _rep in range(reps):
                u_tiles = {}
                # cols 0..16: one dot per quad; the last straddle quad writes
                # two half-range dots to cols 17..18. Host sums all columns.
                acc_all = accp.tile([2, NQ + 1], dt.float32, name="acc_all")


                def dot_range(i, lo, hi, col):
                    # dot u[:, lo:hi] with s_cols into acc_all col `col`
                    scr = spool.tile([2, 512], dt.bfloat16, name="scr")
                    nc.vector.tensor_mul(
                        scr[:, 0:hi - lo], u_tiles[i][:, lo:hi],
                        scols_sb[:, i, lo:hi],
                    )
                    nc.vector.tensor_reduce(
                        out=acc_all[:, col:col + 1],
                        in_=scr[:, 0:hi - lo],
                        axis=mybir.AxisListType.X,
                        op=mybir.AluOpType.add,
                    )

                def consume(unit, e_tile):
                    """mask + u-matmuls + (on quad completion) the DVE dot."""
                    if "umm" in ablate or "dot" in ablate:
                        return
                    for i, j, off in unit:
                        last = i == STRADDLE_SLOTS[-1]
                        if i in STRADDLE_SLOTS:
                            # Diagonal 128x128 subtile computed UNMASKED with
                            # half-weighted s rows: by subtile symmetry this
                            # contributes exactly (strict-upper) + diag/2, and
                            # 2*D' then equals 2*upper + diag — the full loss.
                            # For the last quad the group check is skipped so
                            # the incremental dots may read completed column
                            # ranges mid-group (per-element has_written makes
                            # this safe on silicon).
                            lo = 128 * j
                            nc.tensor.matmul(
                                out=u_tiles[i][:, lo:lo + 128],
                                lhsT=srows_sb[:, i, j, 2:4],
                                rhs=e_tile[:, off: off + 128],
                                start=(j == 0),
                                stop=(j == 3),
                                skip_group_check=last,
                            )
                            if j < 3:
                                nc.tensor.matmul(
                                    out=u_tiles[i][:, lo + 128:512],
                                    lhsT=srows_sb[:, i, j, 0:2],
                                    rhs=e_tile[:, off + 128: off + 512 - lo],
                                    start=False,
                                    stop=False,
                                    skip_group_check=last,
                                )
                        else:
                            nc.tensor.matmul(
                                out=u_tiles[i][:, 0:512],
                                lhsT=srows_sb[:, i, j, 0:2],
                                rhs=e_tile[:, off: off + 512],
                                start=(j == 0),
                                stop=(j == 3),
                            )
                        if i == STRADDLE_SLOTS[-1]:
                            # last quad: u[:, 0:128(j+1)] is final right after
                            # u-mm j, so split its dot in two — the first half
                            # overlaps the last exps and the tail chain is one
                            # short [2,256] mult+reduce.
                            if j == 1:
                                dot_range(i, 0, 256, 17)
                            elif j == 3:
                                dot_range(i, 256, 512, 18)
                                u_tiles.pop(i)
                        elif j == 3:
                            dot_range(i, 0, 512, i)
                            u_tiles.pop(i)

                # Units: full quads stream as 3-block groups of [128,1536];
                # straddle quads 14-16 are packed [128,1280] units interleaved
                # among the full groups. The first three full blocks and the
                # last straddle quad run as single-block units: a short first
                # exp starts ACT sooner, and a short last exp plus incremental
                # dots shrink the serial tail.
                fulls = [(i, j) for i in range(NQ) if i not in STRADDLE_SLOTS
                         for j in range(4)]
                funits = [[(fulls[k][0], fulls[k][1], 0)] for k in range(3)] + [
                    [(i, j, 512 * k) for k, (i, j) in enumerate(fulls[g:g + 3])]
                    for g in range(3, len(fulls), 3)
                ]
                sunits = [
                    [(i, j, _S_OFF[j]) for j in range(4)] for i in STRADDLE_SLOTS[:3]
                ]
                s_last = [[(STRADDLE_SLOTS[-1], j, 0)] for j in range(4)]
                units = (funits[0:7] + sunits[0:1] + funits[7:12] + sunits[1:2]
                         + funits[12:16] + sunits[2:3] + funits[16:] + s_last)
                assert sum(len(u) for u in units) == NB

                # one-unit lag between production (G-mm + exp) and consumption
                # (mask/u-mm/dot) so in-order PE/DVE queues never stall the
                # next unit's matmuls behind a dependency on this unit's E.
                prev = None
                for unit in units:
                    g_t = gpool.tile([128, 3 * 512], dt.float32, name="g_t")
                    width = 0
                    for i, j, off in unit:
                        if j == 0:
                            u_tiles[i] = upool.tile([2, 512], dt.float32, name="u_t")
                        lo = 128 * j if i in STRADDLE_SLOTS else 0
                        nc.tensor.matmul(
                            out=g_t[:, off: off + 512 - lo],
                            lhsT=stat_sb[:, i, j * 128:(j + 1) * 128],
                            rhs=mov_sb[:, i, lo:512],
                            start=True,
                            stop=True,
                        )
                        width = max(width, off + 512 - lo)
                    e_t = epool.tile([128, 3 * 512], dt.bfloat16, name="e_t")
                    if "exp" in ablate:
                        nc.vector.tensor_copy(e_t[:, 0:width], g_t[:, 0:width])
                    else:
                        nc.scalar.activation(
                            out=e_t[:, 0:width],
                            in_=g_t[:, 0:width],
                            func=mybir.ActivationFunctionType.Exp,
                        )
                    if prev is not None:
                        consume(*prev)
                    prev = (unit, e_t)
                consume(*prev)

                nc.sync.dma_start(out=out[:, :], in_=acc_all[:, :])

    nc.compile()
    return nc


def _get_program(reps=1):
    if reps not in _PROGRAM_CACHE:
        _PROGRAM_CACHE[reps] = _build_program(reps)
    return _PROGRAM_CACHE[reps]


def _quad_assignment():
    """Per-image quad lists for the two cores that share an image.
    Straddle quads must land on STRADDLE_SLOTS (the program masks those)."""
    full = [(c, q) for c in range(8) for q in range(c)]  # 28 quads
    stra = [(c, c) for c in range(8)]                    # 8 quads

    def arrange(fulls, stras):
        fi, si = iter(fulls), iter(stras)
        return [next(si) if s in STRADDLE_SLOTS else next(fi) for s in range(NQ)]

    even = arrange(full[0::2], stra[0:4])
    odd = arrange(full[1::2], stra[4:8])
    return even, odd


def _prepare_inputs(images, segmentations):
    """Host-side shard/pack: downsample, build bf16 hi/lo feature stacks,
    lay out per-core canonical quad arrays."""
    N = images.shape[0]
    assert images.shape == (4, 3, 128, 128) and segmentations.shape == (4, 2, 128, 128)

    # nearest resize (scale 0.5) == stride-2 subsample
    img = images[:, :, ::2, ::2].astype(np.float64)  # [4,3,64,64]

    # bilinear resize (scale 0.5, align_corners=False) == 2x2 average pooling;
    # mirror the reference's fp32 evaluation order exactly
    s = segmentations.astype(np.float32)
    t = s[:, :, 0::2, :] * np.float32(0.5) + s[:, :, 1::2, :] * np.float32(0.5)
    seg = t[:, :, :, 0::2] * np.float32(0.5) + t[:, :, :, 1::2] * np.float32(0.5)
    seg = seg.reshape(N, 2, 4096)  # [4,2,P] float32

    sxy = SIGMA_XY * SCALE
    yy, xx = np.meshgrid(np.arange(64.0), np.arange(64.0), indexing="ij")
    pos = np.stack([xx, yy], 0) / sxy  # [2,64,64]
    feats = np.concatenate(
        [np.broadcast_to(pos[None], (N, 2, 64, 64)), img / SIGMA_RGB], axis=1
    )  # [4,5,64,64]
    F = feats.reshape(N, 5, 4096)
    F = F - F.mean(axis=2, keepdims=True)  # translation-invariant; shrinks |f|
    b = -0.5 * (F * F).sum(axis=1)  # [4, P]

    def split(x):
        h = x.astype(_bf16).astype(np.float64)
        l = (x - h).astype(_bf16).astype(np.float64)
        return h, l

    Fh, Fl = split(F)          # [4,5,P] each
    Bh, Bl = split(b)          # [4,P]
    ones = np.ones((N, 1, 4096))

    # stat rows: Fh Fh Fl Fl | Bh Bl 1 1 ; mov rows: Fh Fl Fh Fl | 1 1 Bh Bl
    STAT = np.concatenate(
        [Fh, Fh, Fl, Fl, Bh[:, None], Bl[:, None], ones, ones], axis=1
    ).astype(_bf16)  # [4, 24, P]
    MOV = np.concatenate(
        [Fh, Fl, Fh, Fl, ones, ones, Bh[:, None], Bl[:, None]], axis=1
    ).astype(_bf16)

    seg_bf = seg.astype(_bf16)
    seg_half = (seg * np.float32(0.5)).astype(_bf16)

    even, odd = _quad_assignment()

    in_maps = []
    for core in range(8):
        im = core // 2
        quads = even if core % 2 == 0 else odd
        feat_arr = np.zeros((NROWS, 2, NQ, 512), _bf16)
        srows_arr = np.zeros((128, NQ, 4, 4), _bf16)
        scols_arr = np.zeros((2, NQ, 512), _bf16)
        for slot, (c, q) in enumerate(quads):
            feat_arr[:, 0, slot, :] = STAT[im][:, 512 * q: 512 * (q + 1)]
            feat_arr[:, 1, slot, :] = MOV[im][:, 512 * c: 512 * (c + 1)]
            for j in range(4):
                r = 4 * q + j
                srows_arr[:, slot, j, 0:2] = seg_bf[im][:, 128 * r: 128 * (r + 1)].T
                srows_arr[:, slot, j, 2:4] = seg_half[im][:, 128 * r: 128 * (r + 1)].T
            scols_arr[:, slot, :] = seg_bf[im][:, 512 * c: 512 * (c + 1)]
        in_maps.append(
            {
                "feat": np.ascontiguousarray(feat_arr),
                "srows": np.ascontiguousarray(srows_arr),
                "scols": np.ascontiguousarray(scols_arr),
            }
        )
    return in_maps


def _combine(outs, n_images=4):
    # diag-subtile half-weighting makes 2*sum(core partials) the full
    # quadratic form including the diagonal (see _build_program)
    off = sum(float(o["out"].sum(dtype=np.float64)) for o in outs)
    loss = -WEIGHT * 2.0 * off / n_images
    return np.array([loss], dtype=np.float32)


def kernel(images, segmentations):
    from concourse.bass_utils import run_bass_kernel_spmd

    in_maps = _prepare_inputs(np.asarray(images), np.asarray(segmentations))
    nc = _get_program(reps=1)
    res = run_bass_kernel_spmd(nc, in_maps, core_ids=list(range(8)))
    return _combine(res.results)
